# revision 47
# baseline (speedup 1.0000x reference)
"""MoE top-k routing + grouped down-proj GEMM + reduce-scatter for trn2 (8 cores).

Problem: intermediate_states [4, 2048, 1024] f16 (rank-sharded expanded-token
activations), w [4, 8, 1024, 2048] f16 (rank-sharded per-expert down-proj),
router_logits [1024, 8] f32, topk=2.  Output [4, 256, 2048] f16.

Strategy: per expanded token tk routed to expert e(tk):
y_part[tk] = gate(tk) * (x_full[tk] @ w_full[e(tk)]) with x_full [TK, 4096]
(rank dim folded into the contraction) and w_full[e] [4096, 2048].

Work is decomposed into (expert, K-half) groups; a group's tokens are split
into 32-token chunks (sum over groups = ~136 chunks for balanced routing vs
160 128-token-padded quarters in a 5-job layout).  Each core holds two W
slices (A, B: one (expert, khalf) [2048, 2048] f16 block each) and runs
4.5 tile-equivalents of PE work (~63 us vs 69 us for the 5-job layout):

- 4 full tiles (t0/t1 on A, t2/t3 on B), each 128 tokens = 4 chunks of one
  slice, as plain M=128 matmuls, two tiles interleaved per k-subtile so
  the pair consumes each arriving 512KB W chunk over ~1.7us, tracking the
  DMA stream rate; the last two k-subtiles de-interleave so the first
  tile's eviction overlaps the second's final matmuls.
- a final half tile with 2 more A-chunks H-split across 4x column tiling
  (tile_size=(128,32)) column pairs, so each of its two nf passes
  accumulates a single PSUM bank on the already-resident A slice; pass 0
  evicts + DMAs out under pass 1, leaving a ~1.5us kernel tail.

Chunk capacity per core: 10 on slice A (8 full-tile + 2 half-tile slots),
8 on B.  The host pairs the 16 (e, kh) groups onto cores (largest with
smallest), which fits whenever the largest group is <= 10 chunks and the
9th largest is <= 8.

Input DMAs ride both HWDGE queues (sync + scalar) with W chunks
alternating and x slotted just-in-time, so per-queue cumulative arrival
tracks consumption; all PSUM evictions (fp32 gate applied as per-partition
scale, fp32->f16) ride the vector engine because ring-full backpressure
waits on the DMA-issuing engines would head-of-line-block them.  PE warmup
matmuls cover the DMA lead-in so the HAM clock gate opens before real
work.  No collective: the host sums each token's 4 partial rows.

Fallback: pathological routing (largest group > 10 chunks etc.) uses an
expert-per-core kernel (full K=4096, capacity padded to 128).
"""

import numpy as np

R, T_TOK, TOPK, E = 4, 1024, 2, 8
I_PR, H = 1024, 2048
K = R * I_PR            # 4096 contraction
P = 128
NF = 512                # matmul free-dim (one PSUM bank of fp32)
NH = H // NF            # 4
N_CORES = 8

KH = K // 2             # 2048 per K-half
KS2 = KH // P           # 16 k-subtiles per K-half
CH = 32                 # token chunk granularity (column-tile width)
NFULL = 4               # full tiles per core (+1 half tile)
NTILE = NFULL + 1
CAP_A, CAP_B = 10, 8    # chunk-slot capacity per W slice
# compiled tile -> W slice map for the 4 full tiles (0=A, 1=B); the half
# tile is all slice A and runs last: its 4 column slots are 2 A-chunks
# H-split across column pairs (cols 0/1 = chunk 0's H-halves, 2/3 = chunk
# 1's), so each of its two nf passes accumulates in a single PSUM bank and
# the kernel tail is one bank's eviction + a 128KB output DMA.
SLICE_OF = (0, 0, 1, 1)
NWARM = 46

# fallback (expert-per-core) mode
KSUB = K // P           # 32
CAP_FB = 384            # token capacity per launch in fallback mode

_prog_cache: dict[str, object] = {}


def _new_bacc():
    from concourse import bacc

    return bacc.Bacc(
        "TRN2",
        target_bir_lowering=False,
        debug=False,
        num_devices=N_CORES,
    )


def _build_program_tiles():
    import concourse.mybir as mybir
    import concourse.tile as tile

    f16 = mybir.dt.float16
    f32 = mybir.dt.float32

    nc = _new_bacc()
    # xj[t, p, ks*P + c*CH + m] = x value of tile-t column-chunk c token m at
    # K-row ks*P + p of the chunk's K-half: the SBUF stationary layout.
    xj = nc.declare_dram_parameter("xj", [NFULL, P, KS2 * P], f16, isOutput=False)
    xh = nc.declare_dram_parameter("xh", [P, KS2 * P], f16, isOutput=False)
    wh = nc.declare_dram_parameter("wh", [2, KS2, P, H], f16, isOutput=False)
    gs = nc.declare_dram_parameter("gs", [P, NTILE], f32, isOutput=False)
    ho = nc.declare_dram_parameter("ho", [NFULL, P, H], f16, isOutput=True)
    ho2 = nc.declare_dram_parameter("ho2", [2, P, NF], f16, isOutput=True)

    with tile.TileContext(nc) as tc:
        with tc.tile_pool(name="sb", bufs=1) as sb, \
             tc.tile_pool(name="ps", bufs=2, space="PSUM") as psp:
            xt = [sb.tile([P, KS2 * P], f16, name=f"x{t}", tag=f"x{t}", bufs=1)
                  for t in range(NFULL)]
            xh_t = sb.tile([P, KS2 * P], f16, name="xh", tag="xh", bufs=1)
            wt = [[sb.tile([P, H], f16, name=f"w{s}_{ks}", tag=f"w{s}_{ks}",
                           bufs=1) for ks in range(KS2)] for s in range(2)]
            g_raw = sb.tile([P, NTILE], f32, name="g_raw", tag="g_raw", bufs=1)

            HXB = KS2 * P // 2  # half of a full x tile's free dim

            def dma_x(t, half, eng=None):
                sl = slice(half * HXB, (half + 1) * HXB)
                (eng or nc.sync).dma_start(xt[t][:, sl], xj[t, :, sl])

            def dma_w(s, ks, eng=None):
                (eng or nc.sync).dma_start(wt[s][ks][:], wh[s, ks, :, :])

            # Bulk input DMAs ride BOTH HW queues (sync + scalar), W chunks
            # alternating even/odd so each ring's cumulative arrival (at
            # ~half the aggregate bandwidth) tracks the interleaved
            # consumption order; x tiles are slotted just-in-time.
            HWB = H // 2
            QXB = KS2 * P // 4  # quarter of a full x tile's free dim

            def dma_xq(t, quarter, eng):
                sl = slice(quarter * QXB, (quarter + 1) * QXB)
                eng.dma_start(xt[t][:, sl], xj[t, :, sl])

            # sync queue: wA0/wA1 first halves, t0 x quarters, even W
            # chunks, x2.
            nc.sync.dma_start(g_raw[:], gs[:, :])
            nc.sync.dma_start(wt[0][0][:, :HWB], wh[0, 0, :, :HWB])
            dma_xq(0, 0, nc.sync)
            nc.sync.dma_start(wt[0][1][:, :HWB], wh[0, 1, :, :HWB])
            dma_w(0, 2)
            dma_xq(0, 1, nc.sync)
            dma_w(0, 4)
            dma_xq(0, 2, nc.sync)
            dma_w(0, 6)
            dma_xq(0, 3, nc.sync)
            for ks in range(8, KS2, 2):
                dma_w(0, ks)
            dma_x(2, 0)
            dma_x(2, 1)
            for ks in range(0, KS2, 2):
                dma_w(1, ks)
            # scalar queue: wA0/wA1 second halves, t1 x quarters, odd W
            # chunks, x3, then the late-needed xh.
            nc.scalar.dma_start(wt[0][0][:, HWB:], wh[0, 0, :, HWB:])
            dma_xq(1, 0, nc.scalar)
            nc.scalar.dma_start(wt[0][1][:, HWB:], wh[0, 1, :, HWB:])
            dma_xq(1, 1, nc.scalar)
            dma_w(0, 3, nc.scalar)
            dma_xq(1, 2, nc.scalar)
            dma_w(0, 5, nc.scalar)
            dma_xq(1, 3, nc.scalar)
            for ks in range(7, KS2, 2):
                dma_w(0, ks, nc.scalar)
            dma_x(3, 0, nc.scalar)
            dma_x(3, 1, nc.scalar)
            for ks in range(1, KS2, 2):
                dma_w(1, ks, nc.scalar)
            nc.scalar.dma_start(xh_t[:], xh[:, :])



            psq = {}

            def open_tile(t):
                psq[t] = [psp.tile([P, NF], f32, name=f"ps{t}_{nf}", tag="ps",
                                   bufs=8) for nf in range(NH)]

            open_tile(0)
            open_tile(1)

            # HAM warmup in the same (128, 32) tile mode as the real matmuls:
            # keeps the PE busy while the first DMAs stream in; garbage goes
            # to tile 0's first PSUM quarter, cleared by the first real
            # start=True matmul.
            warm_in = sb.tile([P, P], f16, name="warm_in", tag="warm", bufs=1)
            nc.vector.memset(warm_in[:], 0.0)
            for i in range(NWARM):
                nc.tensor.matmul(
                    psq[0][0][:, 0:P],
                    lhsT=warm_in[:],
                    rhs=warm_in[:],
                    start=(i == 0),
                    stop=(i == NWARM - 1),
                )

            # full tiles: all 4 column chunks share one W slice, so each
            # (ks, nf) step is a single M=128 matmul (plain 128x128 mode).
            def mm_group(t, ks):
                s = SLICE_OF[t]
                for nf in range(NH):
                    nc.tensor.matmul(
                        psq[t][nf][:, :],
                        lhsT=xt[t][:, ks * P:(ks + 1) * P],
                        rhs=wt[s][ks][:, nf * NF:(nf + 1) * NF],
                        start=(ks == 0),
                        stop=(ks == KS2 - 1),
                    )

            def evict_quarter(t, nf, o_t):
                # all evictions ride the vector engine: the scalar and sync
                # engines issue DMAs, and HWDGE ring-full backpressure waits
                # on those issues would head-of-line-block any eviction
                # emitted behind them.
                nc.vector.tensor_scalar_mul(
                    o_t[:, nf * NF:(nf + 1) * NF], psq[t][nf][:],
                    g_raw[:, t:t + 1])

            def dma_out(t, o_t, half):
                nc.sync.dma_start(
                    ho[t, :, half * (H // 2):(half + 1) * (H // 2)],
                    o_t[:, half * (H // 2):(half + 1) * (H // 2)])

            def evict_pair(ta, tb):
                o_a = sb.tile([P, H], f16, name=f"o{ta}", tag="o", bufs=NTILE)
                o_b = sb.tile([P, H], f16, name=f"o{tb}", tag="o", bufs=NTILE)
                # ta stopped first (de-interleaved phase tail); pipeline both
                # engines and the output DMAs per half.
                evict_quarter(ta, 0, o_a)
                evict_quarter(ta, 1, o_a)
                dma_out(ta, o_a, 0)
                evict_quarter(ta, 2, o_a)
                evict_quarter(ta, 3, o_a)
                dma_out(ta, o_a, 1)
                evict_quarter(tb, 0, o_b)
                evict_quarter(tb, 1, o_b)
                dma_out(tb, o_b, 0)
                evict_quarter(tb, 2, o_b)
                evict_quarter(tb, 3, o_b)
                dma_out(tb, o_b, 1)

            def phase(ta, tb):
                # interleaved per k-subtile (the pair consumes each arriving
                # W chunk over ~1.8us, matching the DMA stream rate); the
                # last two k-subtiles de-interleave so ta's eviction overlaps
                # tb's final matmuls.
                for ks in range(KS2 - 2):
                    for t in (ta, tb):
                        mm_group(t, ks)
                for t in (ta, tb):
                    for ks in (KS2 - 2, KS2 - 1):
                        mm_group(t, ks)
                evict_pair(ta, tb)

            # phase 1: tiles 0+1 on slice A.
            phase(0, 1)
            # phase 2: tiles 2+3 on slice B.
            open_tile(2)
            open_tile(3)
            phase(2, 3)

            # final half tile on resident slice A: 2 chunks H-split across
            # column pairs, so each of its two nf passes accumulates a
            # single PSUM bank; pass 0 evicts and DMAs out while pass 1
            # computes, leaving a ~1.5us kernel tail.
            for nf2 in range(2):
                q = psp.tile([P, NF], f32, name=f"ps4_{nf2}", tag="ps", bufs=8)
                for ks in range(KS2):
                    for c in range(4):
                        nc.tensor.matmul(
                            q[c * CH:(c + 1) * CH, :],
                            lhsT=xh_t[:, ks * P + c * CH:ks * P + (c + 1) * CH],
                            rhs=wt[0][ks][:, ((c % 2) * 2 + nf2) * NF:
                                          ((c % 2) * 2 + nf2 + 1) * NF],
                            start=(ks == 0),
                            stop=(ks == KS2 - 1),
                            tile_position=(0, c * CH),
                        )
                o_h = sb.tile([P, NF], f16, name=f"oh{nf2}", tag="oh", bufs=2)
                nc.vector.tensor_scalar_mul(o_h[:], q[:], g_raw[:, NFULL:NTILE])
                nc.sync.dma_start(ho2[nf2, :, :], o_h[:])
    nc.finalize()
    return nc


def _build_program_fallback(cap: int):
    import concourse.mybir as mybir
    import concourse.tile as tile

    f16 = mybir.dt.float16
    f32 = mybir.dt.float32
    ntok = cap // P

    nc = _new_bacc()
    xT = nc.declare_dram_parameter("xT", [KSUB, P, cap], f16, isOutput=False)
    wk = nc.declare_dram_parameter("wk", [KSUB, P, H], f16, isOutput=False)
    gs = nc.declare_dram_parameter("gs", [P, ntok], f32, isOutput=False)
    ho = nc.declare_dram_parameter("ho", [ntok, P, H], f16, isOutput=True)

    with tile.TileContext(nc) as tc:
        with tc.tile_pool(name="sb", bufs=1) as sb, \
             tc.tile_pool(name="ps", bufs=2, space="PSUM") as psp:
            xt, wt = [], []
            for k in range(KSUB):
                x_t = sb.tile([P, cap], f16, name=f"x{k}", tag=f"x{k}", bufs=1)
                nc.sync.dma_start(x_t[:], xT[k, :, :])
                w_t = sb.tile([P, H], f16, name=f"w{k}", tag=f"w{k}", bufs=1)
                nc.sync.dma_start(w_t[:], wk[k, :, :])
                xt.append(x_t)
                wt.append(w_t)
            g_raw = sb.tile([P, ntok], f32, name="g_raw", tag="g_raw", bufs=1)
            nc.sync.dma_start(g_raw[:], gs[:, :])
            g2 = sb.tile([P, ntok], f32, name="g2", tag="g2", bufs=1)
            nc.scalar.copy(g2[:], g_raw[:])

            for t in range(ntok):
                ps = psp.tile([P, H], f32, name=f"ps{t}", tag="ps", bufs=2)
                for k in range(KSUB):
                    lhs = xt[k][:, t * P:(t + 1) * P]
                    for h in range(NH):
                        nc.tensor.matmul(
                            ps[:, h * NF:(h + 1) * NF],
                            lhsT=lhs,
                            rhs=wt[k][:, h * NF:(h + 1) * NF],
                            start=(k == 0),
                            stop=(k == KSUB - 1),
                        )
                o_t = sb.tile([P, H], f16, name=f"o{t}", tag="o", bufs=ntok)
                nc.scalar.activation(
                    o_t[:],
                    ps[:],
                    mybir.ActivationFunctionType.Copy,
                    scale=g2[:, t:t + 1],
                )
                nc.sync.dma_start(ho[t, :, :], o_t[:])
    nc.finalize()
    return nc


def _get_program(key):
    if key not in _prog_cache:
        if key == "tiles":
            _prog_cache[key] = _build_program_tiles()
        else:
            _prog_cache[key] = _build_program_fallback(int(key.split(":")[1]))
    return _prog_cache[key]


def _route(logits, topk):
    """numpy replica of jax.lax.top_k + softmax over selected logits."""
    idx = np.argsort(-logits, axis=-1, kind="stable")[:, :topk]      # [T, topk]
    vals = np.take_along_axis(logits, idx, axis=-1)
    mx = vals.max(-1, keepdims=True)
    gate = np.exp(vals - mx)
    gate = gate / gate.sum(-1, keepdims=True)                        # f32
    return idx, gate


def _pair_groups(chunk_counts):
    """Pair the 16 (e, kh) groups onto 8 cores: i-th largest with i-th
    smallest.  Returns [(groupA, groupB)] or None if some pair exceeds the
    compiled (CAP_A, CAP_B) chunk-slot capacity."""
    groups = []
    for e, n in enumerate(chunk_counts):
        for kh in range(2):
            groups.append((int(n), e, kh))
    groups.sort(reverse=True)
    pairs = []
    for i in range(N_CORES):
        ga, gb = groups[i], groups[15 - i]
        if ga[0] > CAP_A or gb[0] > CAP_B:
            return None
        pairs.append((ga, gb))
    return pairs


def prepare(inputs):
    """Host routing + per-core input construction.

    Returns (nc, launches, combine): launches is a list of per-launch in_maps
    (one dict per core); combine(list_of_per_launch_results) -> final output.
    """
    x = np.asarray(inputs["intermediate_states"])          # [R, TK, I_PR] f16
    w = np.asarray(inputs["w"])                            # [R, E, I_PR, H] f16
    logits = np.asarray(inputs["router_logits"]).astype(np.float32)  # [T, E]
    topk = int(np.asarray(inputs["topk"]))

    T, E_ = logits.shape
    TK = T * topk
    assert x.shape == (R, TK, I_PR) and w.shape == (R, E_, I_PR, H) and E_ == E

    idx, gate = _route(logits, topk)
    flat_e = idx.reshape(-1)                               # expert of tk
    counts = np.bincount(flat_e, minlength=E)
    starts = np.zeros(E + 1, np.int64)
    starts[1:] = np.cumsum(counts)
    order = np.argsort(flat_e, kind="stable")              # tks sorted by expert
    g_flat = gate.reshape(TK)
    xf = np.ascontiguousarray(x.transpose(1, 0, 2)).reshape(TK, K)  # [TK, 4096]

    chunk_counts = [-(-int(c) // CH) for c in counts]
    pairs = _pair_groups(chunk_counts)
    if pairs is not None:
        return _prepare_tiles(w, xf, g_flat, order, starts, pairs, topk, T)
    return _prepare_fallback(w, xf, g_flat, order, starts, counts, topk, T)


# chunk-slot order per W slice: (tile, col) positions; A overflows into the
# half tile (2 chunks, k-split across column pairs)
A_SLOTS = [(0, 0), (0, 1), (0, 2), (0, 3), (1, 0), (1, 1), (1, 2), (1, 3)]
B_SLOTS = [(2, 0), (2, 1), (2, 2), (2, 3), (3, 0), (3, 1), (3, 2), (3, 3)]


def _prepare_tiles(w, xf, g_flat, order, starts, pairs, topk, T):
    TK = T * topk
    nc = _get_program("tiles")

    CROWS = NFULL * P + 2 * CH  # assembled partial rows per core

    xjs = np.zeros((N_CORES, NFULL, P, KS2, P), np.float16)
    xhs = np.zeros((N_CORES, P, KS2, P), np.float16)
    whs = np.zeros((N_CORES, 2, KS2, P, H), np.float16)
    gss = np.zeros((N_CORES, P, NTILE), np.float32)
    # pos[kh, tk] = row index of tk's kh partial in the assembled h rows
    pos = np.zeros((2, TK), np.int64)

    for core, (ga, gb) in enumerate(pairs):
        for s, (nch, e, kh) in enumerate((ga, gb)):
            if nch == 0:
                continue
            toks_e = order[starts[e]:starts[e + 1]]
            whs[core, s] = np.ascontiguousarray(
                w[2 * kh:2 * kh + 2, e].reshape(KH, H)).reshape(KS2, P, H)
            slots = A_SLOTS if s == 0 else B_SLOTS
            for j in range(nch):
                toks = toks_e[j * CH:(j + 1) * CH]
                n = len(toks)
                xs = xf[toks, kh * KH:(kh + 1) * KH]       # [n, 2048] f16
                blk = xs.reshape(n, KS2, P).transpose(2, 1, 0)  # [P, ks, n]
                if j < len(slots):
                    t, c = slots[j]
                    xjs[core, t, :, :, c * CH:c * CH + n] = blk
                    gss[core, c * CH:c * CH + n, t] = g_flat[toks]
                    pos[kh, toks] = core * CROWS + t * P + c * CH + np.arange(n)
                else:
                    # half tile: chunk jj occupies column pair (2jj, 2jj+1),
                    # same stationary in both (they cover different H halves)
                    jj = j - len(slots)
                    assert s == 0 and jj < 2
                    for c in (2 * jj, 2 * jj + 1):
                        xhs[core, :, :, c * CH:c * CH + n] = blk
                        gss[core, c * CH:c * CH + n, NFULL] = g_flat[toks]
                    pos[kh, toks] = \
                        core * CROWS + NFULL * P + jj * CH + np.arange(n)

    launches = [[{"xj": xjs[c].reshape(NFULL, P, KS2 * P),
                  "xh": xhs[c].reshape(P, KS2 * P),
                  "wh": whs[c], "gs": gss[c]} for c in range(N_CORES)]]

    def combine(all_results):
        res = all_results[0]
        rows = []
        for c in range(N_CORES):
            rows.append(res[c]["ho"].reshape(NFULL * P, H))
            ho2 = res[c]["ho2"]                  # [2 (nf2), P, NF]
            hh = np.empty((2 * CH, H), np.float16)
            for jH in range(4):
                blkrows = ho2[jH % 2].reshape(4, CH, NF)
                hh[0:CH, jH * NF:(jH + 1) * NF] = blkrows[jH // 2]
                hh[CH:2 * CH, jH * NF:(jH + 1) * NF] = blkrows[2 + jH // 2]
            rows.append(hh)
        h_all = np.concatenate(rows, axis=0)
        y = np.zeros((T, H), np.float32)
        for kh in range(2):
            for kk in range(topk):
                y += h_all[pos[kh, kk::topk]].astype(np.float32)
        return y.astype(np.float16).reshape(R, T // R, H)

    return nc, launches, combine


def _prepare_fallback(w, xf, g_flat, order, starts, counts, topk, T):
    TK = T * topk
    cap_needed = -(-max(int(counts.max()), 1) // P) * P
    cap_launch = min(cap_needed, CAP_FB)
    n_launch = -(-cap_needed // cap_launch)
    cap_total = n_launch * cap_launch
    ntok_l = cap_launch // P

    nc = _get_program(f"fb:{cap_launch}")

    pos = np.empty(TK, np.int64)
    for e in range(E):
        toks = order[starts[e]:starts[e + 1]]
        pos[toks] = e * cap_total + np.arange(len(toks))

    launches = []
    for j in range(n_launch):
        in_maps = []
        for e in range(E):
            toks = order[starts[e]:starts[e + 1]][j * cap_launch:(j + 1) * cap_launch]
            c = len(toks)
            xTe = np.zeros((K, cap_launch), np.float16)
            gse = np.zeros((cap_launch,), np.float32)
            if c:
                xTe[:, :c] = xf[toks].T
                gse[:c] = g_flat[toks]
            in_maps.append({
                "xT": np.ascontiguousarray(xTe.reshape(KSUB, P, cap_launch)),
                "wk": np.ascontiguousarray(w[:, e].reshape(K, H)).reshape(KSUB, P, H),
                "gs": np.ascontiguousarray(gse.reshape(ntok_l, P).T),
            })
        launches.append(in_maps)

    def combine(all_results):
        h_all = np.empty((E * cap_total, H), np.float16)
        for j, res in enumerate(all_results):
            for e in range(E):
                h_all[e * cap_total + j * cap_launch:
                      e * cap_total + (j + 1) * cap_launch] = \
                    res[e]["ho"].reshape(cap_launch, H)
        y = h_all[pos[0::topk]].astype(np.float32)
        for kk in range(1, topk):
            y += h_all[pos[kk::topk]].astype(np.float32)
        return y.astype(np.float16).reshape(R, T // R, H)

    return nc, launches, combine


def kernel(**inputs) -> np.ndarray:
    nc, launches, combine = prepare(inputs)
    from concourse.bass_utils import run_bass_kernel_spmd

    all_results = []
    for in_maps in launches:
        res = run_bass_kernel_spmd(nc, in_maps, core_ids=list(range(N_CORES)))
        all_results.append(res.results)
    return combine(all_results)
